# revision 27
# baseline (speedup 1.0000x reference)
"""Fused multi-head attention with dropout for Trainium2 (Bass/Tile), 8-core SPMD.

Problem: out = dropout(softmax(Q @ K^T * scale)) @ V
  Q/K/V: [64, 2048, 64] fp32, dropout_mask: [64, 2048, 2048] fp32, p = 0.5.

Sharding: the 64 batch*heads are split across 8 NeuronCores (8 heads/core),
no cross-device communication.

Per-head device algorithm (head-local, S = 2048, D = 64):
  Scores are computed TRANSPOSED, S^T[k, q] = K @ Q^T, so softmax rows (over
  k) land on the partition axis and the PV product needs no on-chip transpose:
  O^T[d, q] = sum_k V[k, d] * P[k, q] accumulates in PSUM.

  Work is tiled as (head, 512-wide q-window) blocks; each [128, 1024] fp32
  PSUM score tile holds a PAIR of k-chunks over the window (two 512-col QK
  matmuls), which keeps Act at one 1024-wide exp per pair (the per-instr
  overhead floor) while shrinking the O^T/denominator accumulators to
  [64, 512] = 1 PSUM bank each. That lets BOTH double-buffer
  (pst 2x2 + oacc 2x1 + oden 2x1 = 8 banks), so a window's fold/recip/out
  tail never blocks the next window's matmuls - the boundary stall that
  capped earlier layouts.

  Engine split:
   - Act: exp only (PSUM fp32 -> SBUF bf16).
   - PE:  QK (fp32r), PV (bf16), denominator ones-matmuls for PE_PAIRS
          chunk-pairs, and end-of-window accumulator folds.
   - DVE: dropout mask-mult as all-bf16 tensor_tensor (the 2x_1p DVE mode
          needs every operand 2-byte; any u8-mixed op runs 7x slower on HW),
          two bf16 pair-sum accumulators, reciprocal + output multiply
          (deferred one window so they never sit on the critical path).
   - GpSimd: nothing (a dependent gpsimd op costs ~10us pipeline latency,
          and even its software-DGE DMAs measure ~20us slower end-to-end).
  Masks ship as bf16 {0,1}; the 1/(1-p)=2 dropout rescale is folded into the
  0.5-valued ones weights: out = oacc / (0.5 * sum_k exp).
"""

import numpy as np
from contextlib import ExitStack

import concourse.bass as bass
import concourse.bacc as bacc
import concourse.tile as tile
import concourse.mybir as mybir
from concourse.bass_utils import run_bass_kernel_spmd

N_CORES = 8
B, S, D = 64, 2048, 64
HPC = B // N_CORES  # heads per core
KP = 128            # k-chunk size (PSUM partition dim)
NW = 512            # q-window width (one PSUM bank of fp32)
DROP_P = 0.5
N_PAIR = S // (2 * KP)  # 8 chunk-pairs per window

# Chunk-pairs whose denominator ones-matmuls run directly on PE (the rest
# are summed in bf16 on DVE across two accumulators, folded at window end).
PE_PAIRS = (0, 1)
MK_LEAD = 4         # mask DMA prefetch, in pairs


def build_program(
    n_heads=HPC,
    seq=S,
    d=D,
    scale=1.0,
    reps=1,
    pe_pairs=PE_PAIRS,
    mk_lead=MK_LEAD,
):
    f32 = mybir.dt.float32
    bf16 = mybir.dt.bfloat16
    # float32r: same fp32 bytes, PE streams 1 col/cycle (vs 4 for fp32) at
    # ~tf32 precision (HW-probed maxabs 5.8e-3 on N(0,64) scores).
    fmm = mybir.dt.float32r
    n_pair = seq // (2 * KP)
    n_w = seq // NW
    pe_set = set(p for p in pe_pairs if p < n_pair)
    acc_pairs = [p for p in range(n_pair) if p not in pe_set]
    acc_of = {}
    for i, p in enumerate(acc_pairs):
        acc_of[p] = 0 if i < (len(acc_pairs) + 1) // 2 else 1

    nc = bacc.Bacc("TRN2", target_bir_lowering=False, debug=False)
    qt_d = nc.dram_tensor("qt", [n_heads, d, seq], fmm, kind="ExternalInput").ap()
    kt_d = nc.dram_tensor("kt", [n_heads, d, seq], fmm, kind="ExternalInput").ap()
    vp_d = nc.dram_tensor(
        "vp", [n_heads, KP, (seq // KP) * d], bf16, kind="ExternalInput"
    ).ap()
    # mask pre-packed on host into paired-tile layout: row block (w*n_pair+p)
    # holds [128, 1024] = (chunk 2p | chunk 2p+1) over q-window w, so every
    # mask DMA is one contiguous-2KB-row [128, 1024] transfer.
    mt_d = nc.dram_tensor(
        "mt", [n_heads, (seq // NW) * (seq // (2 * KP)) * KP, 2 * NW],
        bf16, kind="ExternalInput",
    ).ap()
    ot_d = nc.dram_tensor("ot", [n_heads, d, seq], f32, kind="ExternalOutput").ap()

    # flat block list: (head, q-window); software-pipelined emission with a
    # global mask-DMA cursor and early head prefetch.
    blocks = [(h, w) for h in range(n_heads) for w in range(n_w)] * reps

    with tile.TileContext(nc) as tc:
        with ExitStack() as ctx:
            const = ctx.enter_context(tc.tile_pool(name="const", bufs=1))
            qkv = ctx.enter_context(tc.tile_pool(name="qkv", bufs=3))
            mpool = ctx.enter_context(tc.tile_pool(name="mask", bufs=10))
            ppool = ctx.enter_context(tc.tile_pool(name="p", bufs=6))
            dpool = ctx.enter_context(tc.tile_pool(name="pd", bufs=4))
            apool = ctx.enter_context(tc.tile_pool(name="acc", bufs=4))
            opool = ctx.enter_context(tc.tile_pool(name="o", bufs=4))
            # PSUM (8 banks): st-pair 2x2, oacc 2x1, oden 2x1
            pst = ctx.enter_context(
                tc.tile_pool(name="pst", bufs=2, space=bass.MemorySpace.PSUM)
            )
            pacc = ctx.enter_context(
                tc.tile_pool(name="pacc", bufs=2, space=bass.MemorySpace.PSUM)
            )
            pden = ctx.enter_context(
                tc.tile_pool(name="pden", bufs=2, space=bass.MemorySpace.PSUM)
            )

            # d identical 0.5-columns: the denominator matmul emits 0.5*sum_k
            # replicated across the d output partitions; the 0.5 folds the
            # dropout 1/(1-p)=2 rescale into the final reciprocal.
            ones = const.tile([KP, d], bf16)
            nc.vector.memset(ones[:], 0.5)

            head_tiles: dict = {}

            def load_head(h):
                qt_sb = qkv.tile([d, seq], fmm, tag="qt")
                nc.sync.dma_start(qt_sb[:], qt_d[h])
                kt_sb = qkv.tile([d, seq], fmm, tag="kt")
                nc.sync.dma_start(kt_sb[:], kt_d[h])
                v_sb = qkv.tile([KP, (seq // KP) * d], bf16, tag="v")
                nc.sync.dma_start(v_sb[:], vp_d[h])
                head_tiles[h] = (qt_sb, kt_sb, v_sb)

            mk_tiles: dict = {}
            st_tiles: dict = {}

            def dma_mk(b, p):
                h, w = blocks[b]
                r0 = (w * n_pair + p) * KP
                t = mpool.tile([KP, 2 * NW], bf16, tag="mk")
                nc.sync.dma_start(t[:], mt_d[h, r0 : r0 + KP, :])
                mk_tiles[(b, p)] = t

            def qk(b, p):
                h, w = blocks[b]
                q0 = w * NW
                qt_sb, kt_sb, _ = head_tiles[h]
                t = pst.tile([KP, 2 * NW], f32, tag="st")
                for i in (0, 1):
                    k0 = (2 * p + i) * KP
                    nc.tensor.matmul(
                        t[:, i * NW : (i + 1) * NW],
                        kt_sb[:, k0 : k0 + KP],
                        qt_sb[:, q0 : q0 + NW],
                        start=True,
                        stop=True,
                    )
                st_tiles[(b, p)] = t

            mk_sched = [(bb, pp) for bb in range(len(blocks)) for pp in range(n_pair)]
            mk_cursor = [0]

            def advance_mk(n):
                for _ in range(n):
                    if mk_cursor[0] < len(mk_sched):
                        dma_mk(*mk_sched[mk_cursor[0]])
                        mk_cursor[0] += 1

            load_head(0)
            advance_mk(mk_lead)
            qk(0, 0)

            n_dsrc = 2 * len(pe_set) + 2 * min(2, max(1, len(acc_pairs)))
            pending = [None, None]  # deferred out-stage compute / dma

            for b, (h, w) in enumerate(blocks):
                _, _, v_sb = head_tiles[h]
                oacc = pacc.tile([d, NW], f32, tag="oacc")
                oden = pden.tile([d, NW], f32, tag="oden")
                accs = [None, None]
                pend = [None, None]  # first p0-pair of an accumulator
                dsrc = [0]

                def oden_fold(src, oden=oden, dsrc=dsrc):
                    # one ones-matmul per 512-col half, accumulating into oden
                    for i in (0, 1):
                        nc.tensor.matmul(
                            oden[:],
                            ones,
                            src[:, i * NW : (i + 1) * NW],
                            start=dsrc[0] == 0,
                            stop=dsrc[0] == n_dsrc - 1,
                        )
                        dsrc[0] += 1

                for p in range(n_pair):
                    # prefetch the next head's tensors early in this head's
                    # first window (~25us of lead over first use)
                    if (
                        p == n_pair // 2
                        and w == 0
                        and b + n_w < len(blocks)
                        and blocks[b + n_w][0] != h
                    ):
                        load_head(blocks[b + n_w][0])
                    advance_mk(1)

                    st = st_tiles.pop((b, p))
                    p0 = ppool.tile([KP, 2 * NW], bf16, tag="p0")
                    nc.scalar.activation(
                        p0[:], st[:], mybir.ActivationFunctionType.Exp, scale=scale
                    )
                    nxt = (b, p + 1) if p + 1 < n_pair else (b + 1, 0)
                    if nxt[0] < len(blocks):
                        qk(*nxt)
                    mk = mk_tiles.pop((b, p))
                    pd = dpool.tile([KP, 2 * NW], bf16, tag="pd")
                    nc.vector.tensor_tensor(pd[:], mk[:], p0[:], mybir.AluOpType.mult)
                    if p == 0 and pending[0] is not None:
                        pending[1] = pending[0]()
                        pending[0] = None
                    elif p == 1 and pending[1] is not None:
                        pending[1]()
                        pending[1] = None
                    for i in (0, 1):
                        c = 2 * p + i
                        nc.tensor.matmul(
                            oacc[:],
                            v_sb[:, c * d : (c + 1) * d],
                            pd[:, i * NW : (i + 1) * NW],
                            start=p == 0 and i == 0,
                            stop=p == n_pair - 1 and i == 1,
                        )
                    # denominator contribution of this pair
                    if p in pe_set:
                        oden_fold(p0)
                    else:
                        ai = acc_of[p]
                        if accs[ai] is None and pend[ai] is None:
                            pend[ai] = p0
                        elif accs[ai] is None:
                            t = apool.tile([KP, 2 * NW], bf16, tag="acc")
                            nc.vector.tensor_tensor(
                                t[:], pend[ai][:], p0[:], mybir.AluOpType.add
                            )
                            accs[ai] = t
                            pend[ai] = None
                        else:
                            nc.vector.tensor_tensor(
                                accs[ai][:], accs[ai][:], p0[:], mybir.AluOpType.add
                            )

                # fold the bf16 accumulators into the PSUM denominator
                for acc in accs:
                    if acc is not None:
                        oden_fold(acc)
                for pp in pend:
                    if pp is not None:
                        oden_fold(pp)

                # out = oacc * (1 / (0.5 * sum_k exp)); the compute defers to
                # the next window's first mask-mult and the store one further,
                # keeping them off the in-order critical paths.
                def make_out(h=h, w=w, oacc=oacc, oden=oden):
                    def emit():
                        rb = opool.tile([d, NW], f32, tag="rb")
                        nc.vector.reciprocal_approx_fast(rb[:], oden[:])
                        out_sb = opool.tile([d, NW], f32, tag="out")
                        nc.vector.tensor_tensor(
                            out_sb[:], oacc[:], rb[:], mybir.AluOpType.mult
                        )

                        def emit_dma():
                            nc.sync.dma_start(
                                ot_d[h, :, w * NW : (w + 1) * NW], out_sb[:]
                            )

                        return emit_dma

                    return emit

                pending[0] = make_out()
            if pending[0] is not None:
                pending[1] = pending[0]()
            if pending[1] is not None:
                pending[1]()

    nc.compile()
    return nc


_CACHE: dict = {}


def _get_program(scale: float):
    key = float(scale)
    if key not in _CACHE:
        _CACHE[key] = build_program(scale=key)
    return _CACHE[key]


def make_in_maps(query, key, value, dropout_mask, **_ignored):
    """Shard + relayout the full inputs into the 8 per-core input maps."""
    import ml_dtypes

    query = np.asarray(query, dtype=np.float32)
    key = np.asarray(key, dtype=np.float32)
    value = np.asarray(value, dtype=np.float32)
    dropout_mask = np.asarray(dropout_mask, dtype=np.float32)
    in_maps = []
    for cid in range(N_CORES):
        sl = slice(cid * HPC, (cid + 1) * HPC)
        qt = np.ascontiguousarray(query[sl].transpose(0, 2, 1))
        kt = np.ascontiguousarray(key[sl].transpose(0, 2, 1))
        vp = np.ascontiguousarray(
            value[sl].reshape(HPC, S // KP, KP, D).transpose(0, 2, 1, 3)
        ).reshape(HPC, KP, (S // KP) * D).astype(ml_dtypes.bfloat16)
        keep = (dropout_mask[sl].transpose(0, 2, 1) >= DROP_P)  # [h, k, q]
        # pack to paired-tile layout [h, (w*n_pair+p)*KP, 2*NW]
        n_pair = S // (2 * KP)
        n_w = S // NW
        mt = np.ascontiguousarray(
            keep.reshape(HPC, n_pair, 2, KP, n_w, NW).transpose(0, 4, 1, 3, 2, 5)
        ).reshape(HPC, n_w * n_pair * KP, 2 * NW).astype(ml_dtypes.bfloat16)
        in_maps.append({"qt": qt, "kt": kt, "vp": vp, "mt": mt})
    return in_maps


def run(query, key, value, scale_factor, dropout_mask, trace=False, **trace_kwargs):
    scale = float(np.asarray(scale_factor).reshape(()))
    nc = _get_program(scale)
    in_maps = make_in_maps(query, key, value, dropout_mask)
    res = run_bass_kernel_spmd(
        nc, in_maps, core_ids=list(range(N_CORES)), trace=trace, **trace_kwargs
    )
    outs = [res.results[c]["ot"].transpose(0, 2, 1) for c in range(N_CORES)]
    full = np.ascontiguousarray(np.concatenate(outs, axis=0), dtype=np.float32)
    return full, res


def kernel(query, key, value, scale_factor, dropout_mask):
    out, _ = run(query, key, value, scale_factor, dropout_mask, trace=False)
    return out


# revision 28
# speedup vs baseline: 1.1121x; 1.1121x over previous
"""Fused multi-head attention with dropout for Trainium2 (Bass/Tile), 8-core SPMD.

Problem: out = dropout(softmax(Q @ K^T * scale)) @ V
  Q/K/V: [64, 2048, 64] fp32, dropout_mask: [64, 2048, 2048] fp32, p = 0.5.

Sharding: the 64 batch*heads are split across 8 NeuronCores (8 heads/core),
no cross-device communication.

Per-head device algorithm (head-local, S = 2048, D = 64):
  Scores are computed TRANSPOSED, S^T[k, q] = K @ Q^T, so softmax rows (over
  k) land on the partition axis and the PV product needs no on-chip transpose:
  O^T[d, q] = sum_k V[k, d] * P[k, q] accumulates in PSUM.

  Engine balance (the baseline was PE-bound at ~327us/core because the
  softmax denominator sum_k exp(s) was a ones-matmul per k-chunk - 1/3 of
  all PE cycles - with Vector near-saturated and GpSimd idle):
   - Act: exp only ([128,1024] fp32 PSUM tiles -> bf16 SBUF), nothing else;
          at 1.2G rows/s + ~185ns/instr this is the ~266us pipeline pacer.
   - PE:  QK (fp32r) + PV (bf16) + denominator ones-matmuls for PE_CHUNKS
          + accumulator folds at end of slice (end placement matters: a
          fold mid-slice stalls the in-order PE queue on the DVE chain).
   - DVE: dropout mask-mult as all-bf16 tensor_tensor (the 2x_1p DVE mode
          needs every operand 2-byte; HW-measured 682ns/[128,1024] tile vs
          3.7us for any u8-mixed op), two bf16 denominator chunk-sum
          accumulators, reciprocal + final output multiply, both deferred
          into the next slice so they stay off the critical path.
   - GpSimd: nothing (a dependent gpsimd op costs ~10us pipeline latency on
          HW, and even its software-DGE DMAs measure ~20us slower).
  Chunk-major [128,1024] score tiles keep one stationary load per two
  matmuls (a 512-wide variant that halved the accumulator banks measured
  +24us from doubled stationary reloads, unmodeled in CoreSim).
  Masks ship as bf16 {0,1} with a 5-chunk-lead DMA cursor; head tensors
  prefetch ~1.5 slices early so their 7us of DMA never starves the mask
  stream. The 1/(1-p)=2 dropout rescale is folded into the 0.5-valued ones
  weights: out = oacc / (0.5 * sum_k exp).
"""

import numpy as np
from contextlib import ExitStack

import concourse.bass as bass
import concourse.bacc as bacc
import concourse.tile as tile
import concourse.mybir as mybir
from concourse.bass_utils import run_bass_kernel_spmd

N_CORES = 8
B, S, D = 64, 2048, 64
HPC = B // N_CORES  # heads per core
KP = 128            # k-chunk size (PSUM partition dim)
QL = 1024           # q-slice width (one [128,1024] PSUM score tile = 2 banks)
NQ = 512            # matmul moving free-dim tile (one fp32 PSUM bank)
DROP_P = 0.5
N_KC = S // KP      # 16 k-chunks
MK_LEAD = 5         # mask DMA prefetch lead, in chunks

# Chunks whose denominator ones-matmul runs directly on PE (rest are summed
# in bf16 on DVE, two accumulators, folded by PE at end of slice).
PE_CHUNKS = tuple(range(5))


def build_program(
    n_heads=HPC,
    seq=S,
    d=D,
    scale=1.0,
    reps=1,
    pe_chunks=PE_CHUNKS,
    mk_lead=MK_LEAD,
):
    f32 = mybir.dt.float32
    bf16 = mybir.dt.bfloat16
    # float32r: same fp32 bytes, PE streams 1 col/cycle (vs 4 for fp32) at
    # ~tf32 precision (HW-probed maxabs 5.8e-3 on N(0,64) scores).
    fmm = mybir.dt.float32r
    n_kc = seq // KP
    n_qh = seq // QL
    n_j = QL // NQ
    pe_set = set(c for c in pe_chunks if c < n_kc)
    dve_accs = [c for c in range(n_kc) if c not in pe_set]
    acc_of = {}
    for i, c in enumerate(dve_accs):
        acc_of[c] = 0 if i < (len(dve_accs) + 1) // 2 else 1

    nc = bacc.Bacc("TRN2", target_bir_lowering=False, debug=False)
    qt_d = nc.dram_tensor("qt", [n_heads, d, seq], fmm, kind="ExternalInput").ap()
    kt_d = nc.dram_tensor("kt", [n_heads, d, seq], fmm, kind="ExternalInput").ap()
    vp_d = nc.dram_tensor("vp", [n_heads, KP, n_kc * d], bf16, kind="ExternalInput").ap()
    mt_d = nc.dram_tensor("mt", [n_heads, seq, seq], bf16, kind="ExternalInput").ap()
    ot_d = nc.dram_tensor("ot", [n_heads, d, seq], f32, kind="ExternalOutput").ap()

    # Software-pipelined emission over a flat list of (head, q-slice) blocks:
    # per chunk c the program order is [dma mask(cursor)] [exp(c)] [QK(next)]
    # [mask-mult(c)] [PV(c)] [denom(c)], so each engine's in-order stream
    # never waits on the current chunk's cross-engine chain.
    blocks = [(h, qh) for h in range(n_heads) for qh in range(n_qh)] * reps

    with tile.TileContext(nc) as tc:
        with ExitStack() as ctx:
            const = ctx.enter_context(tc.tile_pool(name="const", bufs=1))
            qkv = ctx.enter_context(tc.tile_pool(name="qkv", bufs=2))
            mpool = ctx.enter_context(tc.tile_pool(name="mask", bufs=12))
            ppool = ctx.enter_context(tc.tile_pool(name="p", bufs=8))
            dpool = ctx.enter_context(tc.tile_pool(name="pd", bufs=5))
            apool = ctx.enter_context(tc.tile_pool(name="acc", bufs=4))
            opool = ctx.enter_context(tc.tile_pool(name="o", bufs=3))
            # PSUM budget (8 banks): st 2x2 + oacc 2 + oden 2.
            pst = ctx.enter_context(
                tc.tile_pool(name="pst", bufs=2, space=bass.MemorySpace.PSUM)
            )
            pacc = ctx.enter_context(
                tc.tile_pool(name="pacc", bufs=1, space=bass.MemorySpace.PSUM)
            )
            pden = ctx.enter_context(
                tc.tile_pool(name="pden", bufs=1, space=bass.MemorySpace.PSUM)
            )

            # d identical 0.5-columns: the denominator matmul then emits
            # 0.5*sum_k already replicated across the d output partitions,
            # and the 0.5 folds the dropout 1/(1-p)=2 rescale into the
            # final reciprocal.
            ones = const.tile([KP, d], bf16)
            nc.vector.memset(ones[:], 0.5)

            head_tiles: dict = {}

            def load_head(h):
                qt_sb = qkv.tile([d, seq], fmm, tag="qt")
                nc.sync.dma_start(qt_sb[:], qt_d[h])
                kt_sb = qkv.tile([d, seq], fmm, tag="kt")
                nc.sync.dma_start(kt_sb[:], kt_d[h])
                v_sb = qkv.tile([KP, n_kc * d], bf16, tag="v")
                nc.sync.dma_start(v_sb[:], vp_d[h])
                head_tiles[h] = (qt_sb, kt_sb, v_sb)

            mk_tiles: dict = {}
            st_tiles: dict = {}

            def dma_mk(b, c):
                h, qh = blocks[b]
                q0 = qh * QL
                t = mpool.tile([KP, QL], bf16, tag="mk")
                nc.sync.dma_start(t[:], mt_d[h, c * KP : (c + 1) * KP, q0 : q0 + QL])
                mk_tiles[(b, c)] = t

            def qk(b, c):
                h, qh = blocks[b]
                q0 = qh * QL
                qt_sb, kt_sb, _ = head_tiles[h]
                t = pst.tile([KP, QL], f32, tag="st")
                for j in range(n_j):
                    nc.tensor.matmul(
                        t[:, j * NQ : (j + 1) * NQ],
                        kt_sb[:, c * KP : (c + 1) * KP],
                        qt_sb[:, q0 + j * NQ : q0 + (j + 1) * NQ],
                        start=True,
                        stop=True,
                    )
                st_tiles[(b, c)] = t

            mk_sched = [(bb, cc) for bb in range(len(blocks)) for cc in range(n_kc)]
            mk_cursor = [0]

            def advance_mk(n):
                for _ in range(n):
                    if mk_cursor[0] < len(mk_sched):
                        dma_mk(*mk_sched[mk_cursor[0]])
                        mk_cursor[0] += 1

            load_head(0)
            advance_mk(mk_lead)
            qk(0, 0)

            pe_sorted = sorted(pe_set)
            half = (len(dve_accs) + 1) // 2
            n_dsrc = (
                len(pe_sorted) + (1 if half else 0) + (1 if len(dve_accs) - half else 0)
            )

            pending = [None, None]  # deferred out-stage compute / dma

            for b, (h, qh) in enumerate(blocks):
                _, _, v_sb = head_tiles[h]
                oacc = pacc.tile([d, QL], f32, tag="oacc")
                oden = pden.tile([d, QL], f32, tag="oden")
                accs = [None, None]
                pend = [None, None]  # first p0 of an accumulator pair
                dsrc = [0]

                def oden_fold(src, oden=oden, dsrc=dsrc):
                    for j in range(n_j):
                        nc.tensor.matmul(
                            oden[:, j * NQ : (j + 1) * NQ],
                            ones,
                            src[:, j * NQ : (j + 1) * NQ],
                            start=dsrc[0] == 0,
                            stop=dsrc[0] == n_dsrc - 1,
                        )
                    dsrc[0] += 1

                for c in range(n_kc):
                    # prefetch the next head's tensors halfway through this
                    # head's FIRST slice (~17us lead over first use)
                    if (
                        c == n_kc // 2
                        and qh == 0
                        and b + 2 < len(blocks)
                        and blocks[b + 2][0] != h
                    ):
                        load_head(blocks[b + 2][0])
                    advance_mk(1)

                    st = st_tiles.pop((b, c))
                    p0 = ppool.tile([KP, QL], bf16, tag="p0")
                    nc.scalar.activation(
                        p0[:], st[:], mybir.ActivationFunctionType.Exp, scale=scale
                    )
                    nxt = (b, c + 1) if c + 1 < n_kc else (b + 1, 0)
                    if nxt[0] < len(blocks):
                        qk(*nxt)
                    mk = mk_tiles.pop((b, c))
                    pd = dpool.tile([KP, QL], bf16, tag="pd")
                    nc.vector.tensor_tensor(pd[:], mk[:], p0[:], mybir.AluOpType.mult)
                    if c == 0 and pending[0] is not None:
                        pending[1] = pending[0]()
                        pending[0] = None
                    elif c == 1 and pending[1] is not None:
                        pending[1]()
                        pending[1] = None
                    first, last = c == 0, c == n_kc - 1
                    for j in range(n_j):
                        nc.tensor.matmul(
                            oacc[:, j * NQ : (j + 1) * NQ],
                            v_sb[:, c * d : (c + 1) * d],
                            pd[:, j * NQ : (j + 1) * NQ],
                            start=first,
                            stop=last,
                        )
                    # denominator contribution of this chunk
                    if c in pe_set:
                        oden_fold(p0)
                    else:
                        ai = acc_of[c]
                        if accs[ai] is None and pend[ai] is None:
                            pend[ai] = p0
                        elif accs[ai] is None:
                            t = apool.tile([KP, QL], bf16, tag="acc")
                            nc.vector.tensor_tensor(
                                t[:], pend[ai][:], p0[:], mybir.AluOpType.add
                            )
                            accs[ai] = t
                            pend[ai] = None
                        else:
                            nc.vector.tensor_tensor(
                                accs[ai][:], accs[ai][:], p0[:], mybir.AluOpType.add
                            )

                # fold the two bf16 accumulators into the PSUM denominator
                for acc in accs:
                    if acc is not None:
                        oden_fold(acc)
                for pp in pend:
                    if pp is not None:
                        oden_fold(pp)

                # out = oacc * (1 / (0.5 * sum_k exp)); the compute defers to
                # the next slice's first mask-mult and the store one chunk
                # further, keeping both off the in-order critical paths.
                def make_out(h=h, qh=qh, oacc=oacc, oden=oden):
                    def emit():
                        q0 = qh * QL
                        rb = opool.tile([d, QL], f32, tag="rb")
                        nc.vector.reciprocal_approx_fast(rb[:], oden[:])
                        out_sb = opool.tile([d, QL], f32, tag="out")
                        nc.vector.tensor_tensor(
                            out_sb[:], oacc[:], rb[:], mybir.AluOpType.mult
                        )

                        def emit_dma():
                            nc.sync.dma_start(ot_d[h, :, q0 : q0 + QL], out_sb[:])

                        return emit_dma

                    return emit

                pending[0] = make_out()
            if pending[0] is not None:
                pending[1] = pending[0]()
            if pending[1] is not None:
                pending[1]()

    nc.compile()
    return nc


_CACHE: dict = {}


def _get_program(scale: float):
    key = float(scale)
    if key not in _CACHE:
        _CACHE[key] = build_program(scale=key)
    return _CACHE[key]


def make_in_maps(query, key, value, dropout_mask, **_ignored):
    """Shard + relayout the full inputs into the 8 per-core input maps."""
    import ml_dtypes

    query = np.asarray(query, dtype=np.float32)
    key = np.asarray(key, dtype=np.float32)
    value = np.asarray(value, dtype=np.float32)
    dropout_mask = np.asarray(dropout_mask, dtype=np.float32)
    in_maps = []
    for cid in range(N_CORES):
        sl = slice(cid * HPC, (cid + 1) * HPC)
        qt = np.ascontiguousarray(query[sl].transpose(0, 2, 1))
        kt = np.ascontiguousarray(key[sl].transpose(0, 2, 1))
        vp = np.ascontiguousarray(
            value[sl].reshape(HPC, S // KP, KP, D).transpose(0, 2, 1, 3)
        ).reshape(HPC, KP, (S // KP) * D).astype(ml_dtypes.bfloat16)
        mt = (dropout_mask[sl].transpose(0, 2, 1) >= DROP_P).astype(
            ml_dtypes.bfloat16
        )  # [h, k, q] keep-mask
        in_maps.append({"qt": qt, "kt": kt, "vp": vp, "mt": mt})
    return in_maps


def run(query, key, value, scale_factor, dropout_mask, trace=False, **trace_kwargs):
    scale = float(np.asarray(scale_factor).reshape(()))
    nc = _get_program(scale)
    in_maps = make_in_maps(query, key, value, dropout_mask)
    res = run_bass_kernel_spmd(
        nc, in_maps, core_ids=list(range(N_CORES)), trace=trace, **trace_kwargs
    )
    outs = [res.results[c]["ot"].transpose(0, 2, 1) for c in range(N_CORES)]
    full = np.ascontiguousarray(np.concatenate(outs, axis=0), dtype=np.float32)
    return full, res


def kernel(query, key, value, scale_factor, dropout_mask):
    out, _ = run(query, key, value, scale_factor, dropout_mask, trace=False)
    return out
